# revision 42
# baseline (speedup 1.0000x reference)
"""Trainium2 Bass kernel for nn_DP_CAML_33646773797448 (sparse_attention).

Reference computation (per batch b):
    e      = embed_w[ids[b]]                       # (T, D)
    x      = e.T                                   # (D, T)
    h      = relu(conv1d(x, conv_w, pad=K-1) + b)  # (D, T')  T' = T + K - 1
    s      = U @ h                                 # (L, T')  raw scores
    attn   = softmax(s, axis=-1)
    z      = attn @ h.T                            # (L, D)
    logits = sum_d z * U + fc_bias                 # (L,)

Key identity: logits[l] = sum_t softmax(s)[l,t] * s[l,t] — the z-einsum and
final einsum collapse to a softmax-weighted mean of the raw scores.

Sharding: pure data-parallel over B (B == 8 == n_cores), no collectives.

v7: all matmuls fp8e4 with DoubleRow perf mode (256-row contraction per
instruction; PE streams 512-col matmuls back-to-back at ~216ns):
- conv pairs adjacent filter taps (k, k+1) via an overlapping rhs AP
  ([1,2] stride-1 dim over the shifted x window); 15 DoubleRows per
  (t-block, oc), 1024-col blocks split into 512-col psum-bank subs.
- scores use TWO DoubleRow groups per psum tile (8 x 512-col passes per
  l-tile vs 12 for fp16) with partial fp8 error compensation for U:
  U ~ U8 + q8(U - U8) on 212 of 300 d-dims.  The conv output channels
  are PERMUTED (host-side, with U columns permuted to match) so the 88
  uncompensated dims are the globally lowest-|dU| ones.  Group g0 =
  [U8_c0, U8_c1] x (h0, h1); g1 = [dU8_c0, comb2x] x (h0, h2) where the
  rhs pair strides 2 chunks.  comb2x rows: 0..43 = U8 chunk2, 44..87 =
  dU8 chunk2, 88..127 = dU8 of 40 extra chunk1 dims; the conv weight
  layout duplicates the matching output channels into h2 rows 44..127,
  so the extra compensation costs zero additional engine time.
  Measured logits rel err 1.41e-2 (gate 2e-2).
- scales: x,w pre-scaled by 64 on host; h = 32*relu(...) via the relu
  activation (scale 1/128, bias 32*b); scores psum = 32*s; exp scale
  1/32 and stt scalar 1/32 recover exact stats.
- embedding table fp8 on host; gather fp8; PE-transpose in fp8 (output
  element step 2 per HW requirement), DVE copies to x.
- all 70 ut weight tiles resident in SBUF (35KB/partition), loaded by
  70 per-tile DMAs (a single 35k-descriptor DMA wedges the exec unit).
- tail (t' 2048..2056) raw scores staged to an SBUF strip; stats for
  all 70 tails computed in one batch at the end.
- softmax stats: ACT exp (accum -> den) + DVE scalar_tensor_tensor
  (accum -> num) per 1024-col half; PSUM = 4 rotating 1024-f32 slots.
- conv blocks are emitted INSIDE the gather loop as soon as each block's
  x-columns are resident, so conv overlaps the serial indirect-gather
  chain on the in-order PE stream (the 17 gathers are ~1.1us of SWDGE
  fixed cost each, serial on the gpsimd queue).
NOTE: do NOT interleave conv blocks into the scores loop or split the
scores loop into A/B sweeps — both orderings pass CoreSim but hang the
exec unit on hardware (NRT_EXEC_UNIT_UNRECOVERABLE).  Batched
multi-offset indirect gathers pass CoreSim but return wrong data on HW.
"""

import numpy as np
import ml_dtypes

import concourse.bass as bass
import concourse.tile as tile
from concourse import bacc
from concourse import mybir
from concourse.bass_utils import run_bass_kernel_spmd
from concourse.masks import make_identity

F32 = mybir.dt.float32
F16 = mybir.dt.float16
FP8 = mybir.dt.float8e4
I32 = mybir.dt.int32
E4NP = ml_dtypes.float8_e4m3
DR = mybir.MatmulPerfMode.DoubleRow

# Problem shapes (hardcoded per contract)
VOCAB, L, D, K = 50000, 8921, 300, 10
B, T = 8, 2048
TP = T + K - 1            # 2057 valid conv outputs (t' = 0..2056)
CH = 2064                 # h chunk stride (TP padded to mult of 8)
NTB = 17                  # gather blocks of 128 tokens
T_G = NTB * 128           # 2176 gathered tokens (ids padded with token 0)
DPAR = [128, 128, 44]
NDC = 3
LT = (L + 127) // 128     # 70 l-tiles
D_PAD = NDC * 128         # 384
# conv t'-blocks covering [0, 2064); 512-wide so the first block only
# needs the first 5 gathered token-blocks (earlier conv start)
CONV_BLOCKS = [(0, 512), (512, 512), (1024, 1024), (2048, 16)]
TAIL0, TAILW = 2048, 9
WARM_MMS = 16             # PE warm-up matmuls (~5us cold -> HAM to 8/8)
SC_EXP = 1.0 / 32.0       # undo the h scale in exp / stt

_BUILT = {}


def _build_bass():
    nc = bacc.Bacc("TRN2", target_bir_lowering=False, debug=False)

    ids_d = nc.dram_tensor("ids", [T_G], I32, kind="ExternalInput").ap()
    emb_d = nc.dram_tensor("emb8", [VOCAB, D], FP8, kind="ExternalInput").ap()
    w_d = nc.dram_tensor("w_prep", [NDC, 128, K * D_PAD], FP8, kind="ExternalInput").ap()
    cb_d = nc.dram_tensor("cb_prep", [NDC, 128], F32, kind="ExternalInput").ap()
    ut_d = nc.dram_tensor("ut_prep", [LT, 4, 128, 128], FP8, kind="ExternalInput").ap()
    fcb_d = nc.dram_tensor("fcb_prep", [LT, 128], F32, kind="ExternalInput").ap()
    zx_d = nc.dram_tensor("zeros_x", [128 - DPAR[2], T_G], FP8, kind="ExternalInput").ap()
    out_d = nc.dram_tensor("out", [128, LT], F32, kind="ExternalOutput").ap()

    with tile.TileContext(nc) as tc:
        _kernel_body(tc, ids_d, emb_d, w_d, cb_d, ut_d, fcb_d, zx_d, out_d)
    nc.compile()
    return nc


def _pair2(sl, n, step=1):
    return bass.AP(
        tensor=sl.tensor,
        offset=sl.offset,
        ap=[list(sl.ap)[0], [step, 2], [1, n]],
    )


def _kernel_body(tc, ids_d, emb_d, w_d, cb_d, ut_d, fcb_d, zx_d, out_d):
    nc = tc.nc
    from contextlib import ExitStack

    ctx = ExitStack()
    with ctx:
        persist = ctx.enter_context(tc.tile_pool(name="persist", bufs=1))
        epool = ctx.enter_context(tc.tile_pool(name="epool", bufs=17))
        ppool = ctx.enter_context(tc.tile_pool(name="ppool", bufs=3))
        scpool = ctx.enter_context(tc.tile_pool(name="scpool", bufs=2))
        # PSUM: 4 rotating slots x 4KB (1024 f32) = all 8 banks
        psum = ctx.enter_context(tc.tile_pool(name="psum", bufs=4, space="PSUM"))

        # ---- persistent tiles ----
        ids_sb = persist.tile([128, NTB], I32, name="ids_sb", tag="ids_sb")
        nc.sync.dma_start(out=ids_sb[:], in_=ids_d.rearrange("(n p) -> p n", p=128))

        cb_sb = persist.tile([128, NDC], F32, name="cb_sb", tag="cb_sb")
        nc.sync.dma_start(out=cb_sb[:], in_=cb_d.rearrange("c p -> p c"))

        fcb_sb = persist.tile([128, LT], F32, name="fcb_sb", tag="fcb_sb")
        nc.sync.dma_start(out=fcb_sb[:], in_=fcb_d.rearrange("n p -> p n"))

        ident8 = persist.tile([128, 128], FP8, name="ident8", tag="ident8")
        make_identity(nc, ident8[:])

        warm_w = persist.tile([128, 448], F16, name="warm_w", tag="warm_w")
        nc.gpsimd.memset(warm_w[:], 0.0)

        w_sb = []
        for ic in range(NDC):
            wt = persist.tile([128, K * D_PAD], FP8, name=f"w_sb{ic}", tag=f"w_sb{ic}")
            nc.sync.dma_start(out=wt[:], in_=w_d[ic])
            w_sb.append(wt)

        x_all = persist.tile([128, NDC * T_G], FP8, name="x_all", tag="x_all")
        x3 = x_all[:].rearrange("p (c t) -> p c t", t=T_G)
        nc.sync.dma_start(out=x3[DPAR[2] :, 2, :], in_=zx_d)

        h_all = persist.tile([128, NDC * CH], FP8, name="h_all", tag="h_all")
        h3 = h_all[:].rearrange("p (c t) -> p c t", t=CH)

        # all 70 ut tiles resident (35KB/partition); per-tile DMAs (a single
        # 35840-descriptor DMA wedges the exec unit).  Loads are emitted
        # AFTER the gather loop so they don't contend with the gather/x
        # DMAs during the prologue; they still land well ahead of use.
        ut_all = persist.tile([128, LT * 4 * 128], FP8, name="ut_all", tag="ut_all")
        ut_allv = ut_all[:].rearrange("p (n g l) -> p n g l", g=4, l=128)

        den_all = persist.tile([128, 2 * LT], F32, name="den_all", tag="den_all")
        num_all = persist.tile([128, 2 * LT], F32, name="num_all", tag="num_all")
        tails = persist.tile([128, LT * TAILW], F32, name="tails", tag="tails")

        # ---- PE warm-up: dummy matmuls release the HAM clock throttle ----
        warm_ps = psum.tile([128, 1024], F32, name="warm_ps", tag="ps")
        for i in range(WARM_MMS):
            nc.tensor.matmul(
                out=warm_ps[:, :448], lhsT=warm_w[:, :128], rhs=warm_w[:],
                start=True, stop=True,
            )

        # ---- conv as fp8 DoubleRow matmuls (filter-tap pairs) ----
        # emitted interleaved into the gather loop below, as soon as each
        # block's x-columns are resident, so conv overlaps the serial
        # gather chain on the in-order PE stream
        w4 = [w_sb[ic][:].rearrange("p (k o) -> p k o", o=D_PAD) for ic in range(NDC)]

        def conv_block(bi, ocs=(2, 0, 1)):
            t0, tw = CONV_BLOCKS[bi]
            subs = [(o, min(512, tw - o)) for o in range(0, tw, 512)]
            for oc in ocs:
                ps = psum.tile([128, 1024], F32, name=f"cv{t0}_{oc}", tag="ps")
                nmm = (2 * (K // 2) + 3) * len(subs)
                imm = 0
                for ic in range(2):
                    for kp in range(K // 2):
                        for so, sw in subs:
                            nc.tensor.matmul(
                                out=ps[:, so : so + sw],
                                lhsT=w4[ic][:, 2 * kp : 2 * kp + 2, oc * 128 : (oc + 1) * 128],
                                rhs=_pair2(
                                    x3[:, ic, t0 + so + 2 * kp : t0 + so + 2 * kp + sw], sw
                                ),
                                start=(imm < len(subs)),
                                stop=(imm >= nmm - len(subs)),
                                perf_mode=DR,
                            )
                            imm += 1
                for jp in (0, 2, 4):
                    for so, sw in subs:
                        nc.tensor.matmul(
                            out=ps[:, so : so + sw],
                            lhsT=w4[2][:, jp : jp + 2, oc * 128 : (oc + 1) * 128],
                            rhs=_pair2(
                                x3[:, 2, t0 + so + 2 * jp : t0 + so + 2 * jp + sw], sw,
                                step=2,
                            ),
                            start=(imm < len(subs)),
                            stop=(imm >= nmm - len(subs)),
                            perf_mode=DR,
                        )
                        imm += 1
                nc.scalar.activation(
                    out=h3[:, oc, t0 : t0 + tw],
                    in_=ps[:, :tw],
                    func=mybir.ActivationFunctionType.Relu,
                    bias=cb_sb[:, oc : oc + 1],
                    scale=1.0 / 128.0,
                )

        # ---- embedding gather + fp8 transpose into x ----
        def _stride2(tile_ap, part, coff):
            # fp8 transpose requires output element step 2
            sl = tile_ap[:part, coff : coff + 256]
            return bass.AP(
                tensor=sl.tensor, offset=sl.offset,
                ap=[list(sl.ap)[0], [2, 128]],
            )

        for tb in range(NTB):
            e_t = epool.tile([128, D], FP8, name=f"e_t{tb}", tag="e_t")
            nc.gpsimd.indirect_dma_start(
                out=e_t[:],
                out_offset=None,
                in_=emb_d,
                in_offset=bass.IndirectOffsetOnAxis(ap=ids_sb[:, tb : tb + 1], axis=0),
            )
            # stride-2 fp8 layout: chunk dc at cols dc*256 + 2*j
            tp_ps = psum.tile([128, 768], FP8, name=f"tp{tb}", tag="ps")
            for dc in range(NDC):
                dp = DPAR[dc]
                nc.tensor.transpose(
                    out=_stride2(tp_ps, dp, dc * 256),
                    in_=e_t[:, dc * 128 : dc * 128 + dp],
                    identity=ident8[:],
                )
            sl = tp_ps[:, 0:256]
            nc.vector.tensor_copy(
                out=x3[:, 0:2, tb * 128 : (tb + 1) * 128],
                in_=bass.AP(tensor=sl.tensor, offset=sl.offset,
                            ap=[list(sl.ap)[0], [256, 2], [2, 128]]),
            )
            nc.vector.tensor_copy(
                out=x3[: DPAR[2], 2, tb * 128 : (tb + 1) * 128],
                in_=_stride2(tp_ps, DPAR[2], 512),
            )
            s2 = _stride2(tp_ps, DPAR[2], 512)
            if tb == 0:
                nc.vector.tensor_copy(
                    out=x3[64 : 64 + DPAR[2], 2, 0:127],
                    in_=bass.AP(tensor=s2.tensor, offset=s2.offset + 2,
                                ap=[list(s2.ap)[0], [2, 127]]),
                )
            else:
                nc.vector.tensor_copy(
                    out=x3[64 : 64 + DPAR[2], 2, tb * 128 - 1 : tb * 128 + 127],
                    in_=s2,
                )
            if tb < 5:
                # fillers keep the HAM clock at 8/8 until conv work is ready
                fl_ps = psum.tile([128, 1024], F32, name=f"fl{tb}", tag="ps")
                for _ in range(3):
                    nc.tensor.matmul(
                        out=fl_ps[:, :448], lhsT=warm_w[:, :128], rhs=warm_w[:],
                        start=True, stop=True,
                    )
            # conv blocks as soon as their x-columns are gathered
            if tb == 4:
                conv_block(0)
            elif tb == 8:
                conv_block(1)
            elif tb == 16:
                conv_block(2)
                conv_block(3)

        for lt in range(LT):
            nc.sync.dma_start(
                out=ut_allv[:, lt], in_=ut_d[lt].rearrange("g p l -> p g l")
            )

        # scores: 2 DoubleRow groups. g0: [U8_0, U8_1] x (h0, h1);
        # g1: [dU8_0, comb2x] x (h0, h2) via a 2-chunk-strided rhs pair
        def _hpair02(t0, n):
            sl = h3[:, 0, t0 : t0 + n]
            return bass.AP(
                tensor=sl.tensor,
                offset=sl.offset,
                ap=[list(sl.ap)[0], [2 * CH, 2], [1, n]],
            )

        GROUPS = [(0, 0), (2, None)]
        ut_all4 = ut_all[:].rearrange("p (n g l) -> p n g l", g=4, l=128)

        def score_half(lt, j, ho):
            ut_t = ut_all4[:, lt]
            ps = psum.tile([128, 1024], F32, name=f"s{lt}_{j}", tag="ps")
            for gi, (us, hc) in enumerate(GROUPS):
                for so in (0, 512):
                    rhs = (
                        h3[:, hc : hc + 2, ho + so : ho + so + 512]
                        if hc is not None
                        else _hpair02(ho + so, 512)
                    )
                    nc.tensor.matmul(
                        out=ps[:, so : so + 512],
                        lhsT=ut_t[:, us : us + 2, :],
                        rhs=rhs,
                        start=(gi == 0),
                        stop=(gi == len(GROUPS) - 1),
                        perf_mode=DR,
                    )
            col = 2 * lt + j
            p_t = ppool.tile([128, 1024], F16, name=f"p{lt}_{j}", tag="p_t")
            nc.scalar.activation(
                out=p_t[:],
                in_=ps[:],
                func=mybir.ActivationFunctionType.Exp,
                scale=SC_EXP,
                accum_out=den_all[:, col : col + 1],
            )
            sc_t = scpool.tile([128, 1024], F16, name=f"sc{lt}_{j}", tag="sc_t")
            nc.vector.scalar_tensor_tensor(
                out=sc_t[:],
                in0=p_t[:],
                scalar=SC_EXP,
                in1=ps[:],
                op0=mybir.AluOpType.mult,
                op1=mybir.AluOpType.mult,
                accum_out=num_all[:, col : col + 1],
            )

        def tail_tile(lt):
            ut_t = ut_all4[:, lt]
            ps_t = psum.tile([128, 1024], F32, name=f"tl{lt}", tag="ps")
            for gi, (us, hc) in enumerate(GROUPS):
                rhs = (
                    h3[:, hc : hc + 2, TAIL0 : TAIL0 + TAILW]
                    if hc is not None
                    else _hpair02(TAIL0, TAILW)
                )
                nc.tensor.matmul(
                    out=ps_t[:, :TAILW],
                    lhsT=ut_t[:, us : us + 2, :],
                    rhs=rhs,
                    start=(gi == 0),
                    stop=(gi == len(GROUPS) - 1),
                    perf_mode=DR,
                )
            nc.vector.tensor_copy(
                out=tails[:, lt * TAILW : (lt + 1) * TAILW], in_=ps_t[:, :TAILW]
            )

        for lt in range(LT):
            score_half(lt, 0, 0)
            score_half(lt, 1, 1024)
            tail_tile(lt)

        # ---- tail batch + combine partials, divide, add bias, write out ----
        p_strip = persist.tile([128, LT * TAILW], F32, name="p_strip", tag="p_strip")
        ps_strip = persist.tile([128, LT * TAILW], F32, name="ps_strip", tag="ps_strip")
        den = persist.tile([128, LT], F32, name="den", tag="den")
        num = persist.tile([128, LT], F32, name="num", tag="num")
        tden = persist.tile([128, LT], F32, name="tden", tag="tden")
        tnum = persist.tile([128, LT], F32, name="tnum", tag="tnum")
        rec = persist.tile([128, LT], F32, name="rec", tag="rec")
        logit = persist.tile([128, LT], F32, name="logit", tag="logit")
        nc.scalar.activation(
            out=p_strip[:], in_=tails[:],
            func=mybir.ActivationFunctionType.Exp, scale=SC_EXP,
        )
        nc.vector.scalar_tensor_tensor(
            out=ps_strip[:],
            in0=p_strip[:],
            scalar=SC_EXP,
            in1=tails[:],
            op0=mybir.AluOpType.mult,
            op1=mybir.AluOpType.mult,
        )
        for src, dst in ((p_strip, tden), (ps_strip, tnum)):
            nc.vector.tensor_reduce(
                out=dst[:],
                in_=src[:].rearrange("p (n t) -> p n t", t=TAILW),
                axis=mybir.AxisListType.X,
                op=mybir.AluOpType.add,
            )
        nc.vector.tensor_reduce(
            out=den[:],
            in_=den_all[:].rearrange("p (n t) -> p n t", t=2),
            axis=mybir.AxisListType.X,
            op=mybir.AluOpType.add,
        )
        nc.vector.tensor_reduce(
            out=num[:],
            in_=num_all[:].rearrange("p (n t) -> p n t", t=2),
            axis=mybir.AxisListType.X,
            op=mybir.AluOpType.add,
        )
        nc.vector.tensor_tensor(
            out=den[:], in0=den[:], in1=tden[:], op=mybir.AluOpType.add
        )
        nc.vector.tensor_tensor(
            out=num[:], in0=num[:], in1=tnum[:], op=mybir.AluOpType.add
        )
        nc.vector.reciprocal(out=rec[:], in_=den[:])
        nc.vector.tensor_tensor(
            out=logit[:], in0=num[:], in1=rec[:], op=mybir.AluOpType.mult
        )
        nc.vector.tensor_tensor(
            out=logit[:], in0=logit[:], in1=fcb_sb[:], op=mybir.AluOpType.add
        )
        nc.sync.dma_start(out=out_d, in_=logit[:])


def _q8(x):
    return np.clip(x, -240.0, 240.0).astype(E4NP)


def _prep_inputs(ids, embed_w, conv_w, conv_b, U, fc_bias):
    ids = np.ascontiguousarray(np.asarray(ids, dtype=np.int32))
    embed_w = np.asarray(embed_w, dtype=np.float32)
    conv_w = np.asarray(conv_w, dtype=np.float32)
    conv_b = np.asarray(conv_b, dtype=np.float32)
    U = np.asarray(U, dtype=np.float32)
    fc_bias = np.asarray(fc_bias, dtype=np.float32)

    emb8 = np.ascontiguousarray(_q8(embed_w * 64.0))

    # ---- channel permutation for partial dU compensation (2-group scores):
    # compensated layout positions: chunk0 (128, via slot2), chunk2 (44, via
    # comb2x rows 44..87), chunk1 rows 0..39 (40, via h2-dup rows 88..127).
    # Uncompensated: chunk1 rows 40..127 = the 88 lowest-energy dU dims.
    U8f = _q8(U).astype(np.float32)
    dU8f = _q8(U - U8f).astype(np.float32)
    en = (dU8f**2).sum(axis=0)
    r = np.argsort(-en)
    perm = np.empty(D, np.int64)
    perm[0:128] = r[0:128]
    perm[256:300] = r[128:172]
    perm[128:168] = r[172:212]
    perm[168:256] = r[212:300]
    conv_w = conv_w[perm]
    conv_b = conv_b[perm]
    U8f = U8f[:, perm]
    dU8f = dU8f[:, perm]

    # conv weights -> [ic, i_par, k, o_pad] * 64 (input channels NOT permuted)
    # oc2 rows 44..87 dup chunk2 (rows 0..43); rows 88..127 dup chunk1 rows
    # 0..39 (the 40 extra compensated dims)
    w_prep = np.zeros((NDC, 128, K, D_PAD), np.float32)
    cw = conv_w.transpose(1, 2, 0) * 64.0  # (i, k, o)
    for ic in range(NDC):
        ip = DPAR[ic]
        w_prep[ic, :ip, :, :D] = cw[ic * 128 : ic * 128 + ip]
    w_prep[:, :, :, 2 * 128 + 44 : 2 * 128 + 88] = w_prep[:, :, :, 2 * 128 : 2 * 128 + 44]
    w_prep[:, :, :, 2 * 128 + 88 : 2 * 128 + 128] = w_prep[:, :, :, 128 : 168]
    w2 = np.zeros_like(w_prep[2])
    for j in range(K // 2):
        w2[0:44, j] = w_prep[2, 0:44, 2 * j]
        w2[64:108, j] = w_prep[2, 0:44, 2 * j + 1]
    w_prep[2] = w2
    w_prep = np.ascontiguousarray(_q8(w_prep).reshape(NDC, 128, K * D_PAD))

    cb_prep = np.zeros((NDC, 128), np.float32)
    cb_prep.reshape(-1)[:D] = conv_b * 32.0
    cb_prep[2, 44:88] = cb_prep[2, 0:44]
    cb_prep[2, 88:128] = cb_prep[1, 0:40]

    L_PAD = LT * 128

    def tiled(M):  # [L,D] -> [LT, NDC, 128(d), 128(l)]
        Mp = np.zeros((L_PAD, D_PAD), np.float32)
        Mp[:L, :D] = M
        return Mp.reshape(LT, 128, NDC, 128).transpose(0, 2, 3, 1)

    u8t = tiled(U8f)
    du8t = tiled(dU8f)
    ut_prep = np.zeros((LT, 4, 128, 128), np.float32)
    ut_prep[:, 0] = u8t[:, 0]
    ut_prep[:, 1] = u8t[:, 1]
    ut_prep[:, 2] = du8t[:, 0]
    ut_prep[:, 3, 0:44] = u8t[:, 2, 0:44]
    ut_prep[:, 3, 44:88] = du8t[:, 2, 0:44]
    ut_prep[:, 3, 88:128] = du8t[:, 1, 0:40]
    ut_prep = np.ascontiguousarray(ut_prep.astype(E4NP))

    fcb_prep = np.zeros((LT, 128), np.float32)
    fcb_prep.reshape(-1)[:L] = fc_bias

    common = {
        "zeros_x": np.zeros((128 - DPAR[2], T_G), E4NP),
        "emb8": emb8,
        "w_prep": w_prep,
        "cb_prep": cb_prep,
        "ut_prep": ut_prep,
        "fcb_prep": fcb_prep,
    }
    ids_pad = np.zeros((B, T_G), np.int32)
    ids_pad[:, K - 1 : K - 1 + T] = ids
    in_maps = [dict(common, ids=np.ascontiguousarray(ids_pad[b])) for b in range(B)]
    return in_maps


def get_bass():
    if "nc" not in _BUILT:
        _BUILT["nc"] = _build_bass()
    return _BUILT["nc"]


def kernel(ids, embed_w, conv_w, conv_b, U, fc_bias):
    nc = get_bass()
    in_maps = _prep_inputs(ids, embed_w, conv_w, conv_b, U, fc_bias)
    res = run_bass_kernel_spmd(nc, in_maps, list(range(B))).results
    # out[p, lt] = logits[lt*128 + p]
    return np.stack(
        [res[b]["out"].T.reshape(-1)[:L] for b in range(B)], axis=0
    )
